# revision 18
# baseline (speedup 1.0000x reference)
"""Causal self-attention + cross-attention Trainium2 kernel (8 NeuronCores).

Sharding: head-parallel. 16 heads x 2 batches = 32 (b,h) pairs; core c owns
heads {2c, 2c+1} for both batches (its 128 channels of C=1024). Projections
are column-sliced per core; attention runs fully local per head; the output
projection is row-sliced and the 8 partial [B*T, C] outputs are summed on
the host (no device collectives).

Numerics: float32r matmuls for projections/scores/output (fp32 data rounded
to 11-bit mantissa; products are exact in fp32, PSUM accumulates fp32);
fp16 for the probability side (exp output, V, masks) which unlocks PE
column-tiling and keeps ~5e-4 accuracy. Softmax without max-subtraction
(scores are bounded ~|8| for this problem's distributions), exp on ScalarE
with the 1/sqrt(D) scale folded in, scores computed transposed (ST[k,q]) so
no probability transpose is needed before AV.

Phase B is software-pipelined: score matmuls are issued LOOKAHEAD steps
ahead of the matching AV/denominator matmuls so the (FIFO) PE queue never
stalls waiting for ScalarE's exp.
"""
import sys

sys.path.insert(0, "/opt/trn_rl_repo")

import numpy as np

import concourse.bass as bass
import concourse.tile as tile
from concourse import bacc, mybir
from concourse.bass_utils import run_bass_kernel_spmd

dt = mybir.dt

B, T, TC, C, CC, H, D = 2, 2048, 512, 1024, 512, 16, 64
NCORES = 8
CPC = 128          # channels per core = 2 heads * 64
NT = B * T         # 4096 tokens (batch-major)
NTC = B * TC       # 1024 cross tokens
KT_X = C // 128    # 8 contraction tiles over C
KT_C = CC // 128   # 4 contraction tiles over CC
NCH = NT // 512    # 8 token chunks
NCHC = NTC // 512  # 2 cross token chunks
QC_PER_B = T // 512  # 4 q-chunks per batch
KT_PER_B = T // 128  # 16 k-tiles per batch
LOOKAHEAD = 2      # kt steps issued ahead of their AV in the PE queue


def _round_fp32r(x):
    """Round fp32 array to fp32r (11-bit mantissa, RNE) on the host."""
    u = np.ascontiguousarray(x, np.float32).view(np.uint32).copy()
    u += 0x7FF + ((u >> 12) & 1)
    u &= 0xFFFFF000
    return u.view(np.float32)


def _build(zero_bias=False):
    f32, f32r, f16 = dt.float32, dt.float32r, dt.float16
    nc = bacc.Bacc("TRN2", target_bir_lowering=False, debug=False,
                   enable_asserts=True, num_devices=NCORES)

    xT = nc.dram_tensor("xT", [C, NT], f32r, kind="ExternalInput").ap()
    cT = nc.dram_tensor("cT", [CC, NTC], f32r, kind="ExternalInput").ap()
    wq = nc.dram_tensor("wq", [C, CPC], f32r, kind="ExternalInput").ap()
    wk = nc.dram_tensor("wk", [C, CPC], f32r, kind="ExternalInput").ap()
    wv = nc.dram_tensor("wv", [C, CPC], f32r, kind="ExternalInput").ap()
    wcq = nc.dram_tensor("wcq", [C, CPC], f32r, kind="ExternalInput").ap()
    wck = nc.dram_tensor("wck", [CC, CPC], f32r, kind="ExternalInput").ap()
    wcv = nc.dram_tensor("wcv", [CC, CPC], f32r, kind="ExternalInput").ap()
    wp = nc.dram_tensor("wp", [CPC, C], f32r, kind="ExternalInput").ap()
    bias6 = nc.dram_tensor("bias6", [CPC, 6], f32, kind="ExternalInput").ap()
    maskd = nc.dram_tensor("mask", [128, 128], f16, kind="ExternalInput").ap()
    identd = nc.dram_tensor("ident", [128, 128], f32r, kind="ExternalInput").ap()
    out = nc.dram_tensor("out", [NT, C], f32, kind="ExternalOutput").ap()

    Exp = mybir.ActivationFunctionType.Exp
    SCALE = 0.125  # 1/sqrt(D)

    with tile.TileContext(nc) as tc:
        from contextlib import ExitStack
        with ExitStack() as es:
            persist = es.enter_context(tc.tile_pool(name="persist", bufs=1))
            qT_t = persist.tile([128, NT], f32r, tag="qT")
            kT_t = persist.tile([128, NT], f32r, tag="kT")
            qcT_t = persist.tile([128, NT], f32r, tag="qcT")
            kcT_t = persist.tile([128, NTC], f32r, tag="kcT")
            vn_t = persist.tile([128, (NT // 128) * 130], f16, tag="vn")  # [v_h0|1|v_h1|1] per k-tile
            vcn_t = persist.tile([128, (NTC // 128) * 130], f16, tag="vcn")
            yT2_t = persist.tile([128, NT], f32r, tag="yT2")
            wp_t = persist.tile([128, C], f32r, tag="wp")
            bias_t = persist.tile([128, 6], f32, tag="bias")
            mask_t = persist.tile([128, 128], f16, tag="mask")

            nc.sync.dma_start(out=wp_t[:], in_=wp[:])
            nc.sync.dma_start(out=bias_t[:], in_=bias6[:])
            nc.sync.dma_start(out=mask_t[:], in_=maskd[:])

            ident_t = persist.tile([128, 128], f32r, tag="ident")
            nc.sync.dma_start(out=ident_t[:], in_=identd[:])
            vn_r = vn_t[:].rearrange("p (t c) -> p t c", c=130)
            nc.vector.memset(vn_r[:, :, 64:65], 1.0)
            nc.vector.memset(vn_r[:, :, 129:130], 1.0)
            vcn_r = vcn_t[:].rearrange("p (t c) -> p t c", c=130)
            nc.vector.memset(vcn_r[:, :, 64:65], 1.0)
            nc.vector.memset(vcn_r[:, :, 129:130], 1.0)
            zcolf = persist.tile([1, 128], f32, tag="zcolf")
            nc.vector.memset(zcolf[:], 0.0)
            zcol_t = persist.tile([1, 128], f32r, tag="zcol")
            nc.vector.tensor_copy(zcol_t[:], zcolf[:])
            zrowf = persist.tile([1, 512], f32, tag="zrowf")
            nc.vector.memset(zrowf[:], 0.0)
            zrow_t = persist.tile([1, 512], f32r, tag="zrow")
            nc.vector.tensor_copy(zrow_t[:], zrowf[:])

            # ---------------- Phase A: projections ----------------
            with ExitStack() as esa:
                wpool = esa.enter_context(tc.tile_pool(name="wpool", bufs=1))
                apool = esa.enter_context(tc.tile_pool(name="apool", bufs=2))
                vtpool = esa.enter_context(tc.tile_pool(name="vtpool", bufs=1))
                aps = esa.enter_context(tc.tile_pool(name="aps", bufs=3, space="PSUM"))
                apt = esa.enter_context(tc.tile_pool(name="apt", bufs=3, space="PSUM"))

                wq_t = wpool.tile([128, KT_X, CPC], f32r, tag="wq")
                wk_t = wpool.tile([128, KT_X, CPC], f32r, tag="wk")
                wv_t = wpool.tile([128, KT_X, CPC], f32r, tag="wv")
                wcq_t = wpool.tile([128, KT_X, CPC], f32r, tag="wcq")
                wck_t = wpool.tile([128, KT_C, CPC], f32r, tag="wck")
                wcv_t = wpool.tile([128, KT_C, CPC], f32r, tag="wcv")
                for wdram, wtile in ((wq, wq_t), (wk, wk_t), (wv, wv_t),
                                     (wcq, wcq_t), (wck, wck_t), (wcv, wcv_t)):
                    nc.sync.dma_start(
                        out=wtile[:],
                        in_=wdram.rearrange("(kt p) d -> p kt d", p=128))

                vT_t = vtpool.tile([128, NT], f32r, tag="vT")
                vcT_t = vtpool.tile([128, NTC], f32r, tag="vcT")

                def psum_evict(dst_slice, ps, bcol):
                    if zero_bias:
                        nc.vector.tensor_copy(dst_slice, ps[:])
                    else:
                        nc.vector.tensor_scalar_add(dst_slice, ps[:],
                                                    bias_t[:, bcol:bcol + 1])

                xT_r = xT.rearrange("(kt p) t -> p kt t", p=128)
                for ch in range(NCH):
                    xblk = apool.tile([128, KT_X, 512], f32r, tag="xblk")
                    nc.sync.dma_start(out=xblk[:], in_=xT_r[:, :, ch * 512:(ch + 1) * 512])
                    for wtile, dst, bcol in ((wq_t, qT_t, 0), (wk_t, kT_t, 1),
                                             (wv_t, vT_t, 2), (wcq_t, qcT_t, 3)):
                        ps = aps.tile([128, 512], f32, tag="aps")
                        for kt in range(KT_X):
                            nc.tensor.matmul(ps[:], wtile[:, kt, :], xblk[:, kt, :],
                                             start=(kt == 0), stop=(kt == KT_X - 1))
                        psum_evict(dst[:, ch * 512:(ch + 1) * 512], ps, bcol)

                cT_r = cT.rearrange("(kt p) t -> p kt t", p=128)
                for ch in range(NCHC):
                    cblk = apool.tile([128, KT_C, 512], f32r, tag="cblk")
                    nc.sync.dma_start(out=cblk[:], in_=cT_r[:, :, ch * 512:(ch + 1) * 512])
                    for wtile, dst, bcol in ((wck_t, kcT_t, 4), (wcv_t, vcT_t, 5)):
                        ps = aps.tile([128, 512], f32, tag="aps")
                        for kt in range(KT_C):
                            nc.tensor.matmul(ps[:], wtile[:, kt, :], cblk[:, kt, :],
                                             start=(kt == 0), stop=(kt == KT_C - 1))
                        psum_evict(dst[:, ch * 512:(ch + 1) * 512], ps, bcol)

                # Phase A2: vT/vcT -> natural token-major fp16 layout via PE transpose
                for tt in range(NT // 128):
                    pt = apt.tile([128, 128], f32r, tag="apt")
                    nc.tensor.transpose(pt[:], vT_t[:, tt * 128:(tt + 1) * 128], ident_t[:])
                    nc.vector.tensor_copy(vn_t[:, tt * 130:tt * 130 + 64], pt[:, 0:64])
                    nc.vector.tensor_copy(vn_t[:, tt * 130 + 65:tt * 130 + 129], pt[:, 64:128])
                for tt in range(NTC // 128):
                    pt = apt.tile([128, 128], f32r, tag="apt")
                    nc.tensor.transpose(pt[:], vcT_t[:, tt * 128:(tt + 1) * 128], ident_t[:])
                    nc.vector.tensor_copy(vcn_t[:, tt * 130:tt * 130 + 64], pt[:, 0:64])
                    nc.vector.tensor_copy(vcn_t[:, tt * 130 + 65:tt * 130 + 129], pt[:, 64:128])

            # ---------------- Phase B: attention ----------------
            with ExitStack() as esb:
                bpool = esb.enter_context(tc.tile_pool(name="bpool", bufs=2))
                rdpool = esb.enter_context(tc.tile_pool(name="rdpool", bufs=2, space="DRAM"))
                ypool = esb.enter_context(tc.tile_pool(name="ypool", bufs=3))
                expool = esb.enter_context(tc.tile_pool(name="expool", bufs=8))
                stps = esb.enter_context(tc.tile_pool(name="stps", bufs=2, space="PSUM"))
                yh0ps = esb.enter_context(tc.tile_pool(name="yh0ps", bufs=2, space="PSUM"))
                yh1ps = esb.enter_context(tc.tile_pool(name="yh1ps", bufs=2, space="PSUM"))

                def attn_part(b, qc, qlo, is_self):
                    """One softmax-attention accumulation (self or cross) for a
                    512-wide q chunk of batch b. Both heads' transposed score
                    tiles live in one [128,1024] two-bank PSUM tile so a single
                    exp serves both; V tiles carry a ones column so AV row 64
                    accumulates the softmax denominator. Returns per-head
                    normalized [64,512] tiles (ya, yb)."""
                    nkt = (4 * qc + 4) if is_self else KT_C
                    yh_0 = yh0ps.tile([65, 512], f32, tag="yh0")
                    yh_1 = yh1ps.tile([65, 512], f32, tag="yh1")
                    yh = (yh_0, yh_1)

                    pend = []
                    fidx = [0]

                    def flush_one():
                        ex, off, vsrc, vc0, vc1 = pend.pop(0)
                        first = fidx[0] == 0
                        last = fidx[0] == nkt - 1
                        fidx[0] += 1
                        nc.tensor.matmul(
                            yh[0][:, off:512],
                            vsrc[:, vc0:vc0 + 65],
                            ex[:, off:512],
                            start=first, stop=last)
                        nc.tensor.matmul(
                            yh[1][:, off:512],
                            vsrc[:, vc1:vc1 + 65],
                            ex[:, 512 + off:1024],
                            start=first, stop=last)

                    for kt in range(nkt):
                        if is_self:
                            crossing = kt >= 4 * qc
                            off = (kt - 4 * qc) * 128 if crossing else 0
                            klo = b * T + kt * 128
                            ksrc, qsrc, vsrc = kT_t, qT_t, vn_t
                            vbase = (b * KT_PER_B + kt) * 130
                        else:
                            crossing, off = False, 0
                            klo = b * TC + kt * 128
                            ksrc, qsrc, vsrc = kcT_t, qcT_t, vcn_t
                            vbase = (b * KT_C + kt) * 130
                        st = stps.tile([128, 1024], f32, tag="st")
                        nc.tensor.matmul(
                            st[:, off:512],
                            ksrc[0:64, klo:klo + 128],
                            qsrc[0:64, qlo + off:qlo + 512],
                            start=True, stop=True)
                        nc.tensor.matmul(
                            st[:, 512 + off:1024],
                            ksrc[64:128, klo:klo + 128],
                            qsrc[64:128, qlo + off:qlo + 512],
                            start=True, stop=True)
                        ex = expool.tile([128, 1024], f16, tag="ex")
                        nc.scalar.activation(ex[:, off:512], st[:, off:512],
                                             Exp, scale=SCALE)
                        nc.scalar.activation(ex[:, 512 + off:1024],
                                             st[:, 512 + off:1024],
                                             Exp, scale=SCALE)
                        if crossing:
                            nc.vector.tensor_mul(ex[:, off:off + 128],
                                                 ex[:, off:off + 128], mask_t[:])
                            nc.vector.tensor_mul(ex[:, 512 + off:512 + off + 128],
                                                 ex[:, 512 + off:512 + off + 128],
                                                 mask_t[:])
                        pend.append((ex, off, vsrc, vbase, vbase + 65))
                        if len(pend) > LOOKAHEAD:
                            flush_one()
                    while pend:
                        flush_one()

                    # denominators (row 64 of each head tile) -> DRAM -> [128,8]
                    # so the reciprocal uses all 128 DVE lanes -> broadcast back
                    dcp = bpool.tile([65, 1024], f32, tag="dcp")
                    nc.vector.tensor_copy(dcp[64:65, 0:512], yh[0][64:65, :])
                    nc.vector.tensor_copy(dcp[64:65, 512:1024], yh[1][64:65, :])
                    drd = rdpool.tile([1, 1024], f32, tag="drd")
                    nc.sync.dma_start(out=drd[:], in_=dcp[64:65, :])
                    dsb = bpool.tile([128, 8], f32, tag="dsb")
                    nc.sync.dma_start(
                        out=dsb[:], in_=drd[:].rearrange("a (p e) -> p (a e)", p=128))
                    rsb = bpool.tile([128, 8], f32, tag="rsb")
                    nc.vector.reciprocal(rsb[:], dsb[:])
                    rrd = rdpool.tile([1, 1024], f32, tag="rrd")
                    nc.sync.dma_start(
                        out=rrd[:].rearrange("a (p e) -> p (a e)", p=128), in_=rsb[:])
                    bca = bpool.tile([64, 512], f32, tag="bca")
                    nc.sync.dma_start(out=bca[:],
                                      in_=rrd[0:1, 0:512].to_broadcast((64, 512)))
                    bcb = bpool.tile([64, 512], f32, tag="bcb")
                    nc.sync.dma_start(out=bcb[:],
                                      in_=rrd[0:1, 512:1024].to_broadcast((64, 512)))
                    ya = ypool.tile([64, 512], f32, tag="ya")
                    nc.vector.tensor_mul(ya[:], yh[0][0:64, :], bca[:])
                    yb = ypool.tile([64, 512], f32, tag="yb")
                    nc.vector.tensor_mul(yb[:], yh[1][0:64, :], bcb[:])
                    return ya, yb

                cpool = esb.enter_context(tc.tile_pool(name="cpool", bufs=4))
                for b in range(B):
                    for qc in range(QC_PER_B):
                        qlo = b * T + qc * 512
                        ya_c, yb_c = attn_part(b, qc, qlo, is_self=False)
                        ya_s, yb_s = attn_part(b, qc, qlo, is_self=True)
                        nc.vector.tensor_add(yT2_t[0:64, qlo:qlo + 512],
                                             ya_s[:], ya_c[:])
                        ybsum = ypool.tile([64, 512], f32r, tag="ybsum")
                        nc.vector.tensor_add(ybsum[:], yb_s[:], yb_c[:])
                        # partition shift rows 0-63 -> 64-127 via SBUF-SBUF DMA
                        nc.sync.dma_start(out=yT2_t[64:128, qlo:qlo + 512],
                                          in_=ybsum[:])
                        # output projection for this chunk's four t-tiles
                        for tt in range(qlo // 128, qlo // 128 + 4):
                            for co in range(2):
                                po = stps.tile([128, 512], f32, tag="st")
                                nc.tensor.matmul(
                                    po[:], yT2_t[:, tt * 128:(tt + 1) * 128],
                                    wp_t[:, co * 512:(co + 1) * 512],
                                    start=True, stop=True)
                                so = cpool.tile([128, 512], f32, tag="so")
                                nc.vector.tensor_copy(so[:], po[:])
                                nc.sync.dma_start(
                                    out=out[tt * 128:(tt + 1) * 128,
                                            co * 512:(co + 1) * 512],
                                    in_=so[:])

    nc.compile()
    return nc


_NC_CACHE = {}


def _get_nc(zero_bias=False):
    if zero_bias not in _NC_CACHE:
        _NC_CACHE[zero_bias] = _build(zero_bias)
    return _NC_CACHE[zero_bias]


def make_in_maps(x, cross_input, Wk, bk, Wq, bq, Wv, bv, Wck, bck, Wcq, bcq,
                 Wcv, bcv, Wp, bp):
    """Host-side shard + layout prep. Returns per-core input maps."""
    xT = _round_fp32r(np.asarray(x, np.float32).reshape(NT, C).T)
    cT = _round_fp32r(np.asarray(cross_input, np.float32).reshape(NTC, CC).T)
    mask = np.triu(np.ones((128, 128), np.float32)).astype(np.float16)  # 1 iff kk<=qq
    Wq, Wk, Wv = (np.asarray(w, np.float32) for w in (Wq, Wk, Wv))
    Wcq, Wck, Wcv = (np.asarray(w, np.float32) for w in (Wcq, Wck, Wcv))
    Wp = np.asarray(Wp, np.float32)
    in_maps = []
    for c in range(NCORES):
        sl = slice(c * CPC, (c + 1) * CPC)
        bias6 = np.stack([np.asarray(v, np.float32)[sl] for v in
                          (bq, bk, bv, bcq, bck, bcv)], axis=1)
        in_maps.append({
            "xT": xT, "cT": cT,
            "wq": _round_fp32r(Wq[:, sl]), "wk": _round_fp32r(Wk[:, sl]),
            "wv": _round_fp32r(Wv[:, sl]), "wcq": _round_fp32r(Wcq[:, sl]),
            "wck": _round_fp32r(Wck[:, sl]), "wcv": _round_fp32r(Wcv[:, sl]),
            "wp": _round_fp32r(Wp[sl, :]),
            "bias6": np.ascontiguousarray(bias6),
            "mask": mask,
            "ident": np.eye(128, dtype=np.float32),
        })
    return in_maps


def kernel(**inputs):
    in_maps = make_in_maps(**inputs)
    zb = all(not np.any(np.asarray(inputs[k])) for k in
             ("bq", "bk", "bv", "bcq", "bck", "bcv"))
    nc = _get_nc(zero_bias=zb)
    res = run_bass_kernel_spmd(nc, in_maps, list(range(NCORES)))
    acc = np.zeros((NT, C), np.float64)
    for c in range(NCORES):
        acc += res.results[c]["out"]
    acc += np.asarray(inputs["bp"], np.float32)
    return acc.reshape(B, T, C).astype(np.float32)


if __name__ == "__main__":
    nc = _get_nc()
    print("build + compile OK")


# revision 19
# speedup vs baseline: 1.2945x; 1.2945x over previous
"""Causal self-attention + cross-attention Trainium2 kernel (8 NeuronCores).

Sharding: head-parallel. 16 heads x 2 batches = 32 (b,h) pairs; core c owns
heads {2c, 2c+1} for both batches (its 128 channels of C=1024). Projections
are column-sliced per core; attention runs fully local per head; the output
projection is row-sliced and the 8 partial [B*T, C] outputs are summed on
the host (no device collectives).

Numerics: float32r matmuls for projections/scores/output (fp32 data rounded
to 11-bit mantissa; products are exact in fp32, PSUM accumulates fp32);
fp16 for the probability side (exp output, V, masks) which unlocks PE
column-tiling and keeps ~5e-4 accuracy. Softmax without max-subtraction
(scores are bounded ~|8| for this problem's distributions), exp on ScalarE
with the 1/sqrt(D) scale folded in, scores computed transposed (ST[k,q]) so
no probability transpose is needed before AV.

Phase B is software-pipelined: score matmuls are issued LOOKAHEAD steps
ahead of the matching AV/denominator matmuls so the (FIFO) PE queue never
stalls waiting for ScalarE's exp.
"""
import sys

sys.path.insert(0, "/opt/trn_rl_repo")

import numpy as np

import concourse.bass as bass
import concourse.tile as tile
from concourse import bacc, mybir
from concourse.bass_utils import run_bass_kernel_spmd

dt = mybir.dt

B, T, TC, C, CC, H, D = 2, 2048, 512, 1024, 512, 16, 64
NCORES = 8
CPC = 128          # channels per core = 2 heads * 64
NT = B * T         # 4096 tokens (batch-major)
NTC = B * TC       # 1024 cross tokens
KT_X = C // 128    # 8 contraction tiles over C
KT_C = CC // 128   # 4 contraction tiles over CC
NCH = NT // 512    # 8 token chunks
NCHC = NTC // 512  # 2 cross token chunks
QC_PER_B = T // 512  # 4 q-chunks per batch
KT_PER_B = T // 128  # 16 k-tiles per batch
LOOKAHEAD = 2      # kt steps issued ahead of their AV in the PE queue


def _round_fp32r(x):
    """Round fp32 array to fp32r (11-bit mantissa, RNE) on the host."""
    u = np.ascontiguousarray(x, np.float32).view(np.uint32).copy()
    u += 0x7FF + ((u >> 12) & 1)
    u &= 0xFFFFF000
    return u.view(np.float32)


def _build(zero_bias=False):
    f32, f32r, f16 = dt.float32, dt.float32r, dt.float16
    nc = bacc.Bacc("TRN2", target_bir_lowering=False, debug=False,
                   enable_asserts=True, num_devices=NCORES)

    xT = nc.dram_tensor("xT", [C, NT], f32r, kind="ExternalInput").ap()
    cT = nc.dram_tensor("cT", [CC, NTC], f32r, kind="ExternalInput").ap()
    wq = nc.dram_tensor("wq", [C, CPC], f32r, kind="ExternalInput").ap()
    wk = nc.dram_tensor("wk", [C, CPC], f32r, kind="ExternalInput").ap()
    wv = nc.dram_tensor("wv", [C, CPC], f32r, kind="ExternalInput").ap()
    wcq = nc.dram_tensor("wcq", [C, CPC], f32r, kind="ExternalInput").ap()
    wck = nc.dram_tensor("wck", [CC, CPC], f32r, kind="ExternalInput").ap()
    wcv = nc.dram_tensor("wcv", [CC, CPC], f32r, kind="ExternalInput").ap()
    wp = nc.dram_tensor("wp", [CPC, C], f32r, kind="ExternalInput").ap()
    bias6 = nc.dram_tensor("bias6", [CPC, 6], f32, kind="ExternalInput").ap()
    maskd = nc.dram_tensor("mask", [128, 128], f16, kind="ExternalInput").ap()
    identd = nc.dram_tensor("ident", [128, 128], f32r, kind="ExternalInput").ap()
    out = nc.dram_tensor("out", [NT, C], f32, kind="ExternalOutput").ap()

    Exp = mybir.ActivationFunctionType.Exp
    SCALE = 0.125  # 1/sqrt(D)

    with tile.TileContext(nc) as tc:
        from contextlib import ExitStack
        with ExitStack() as es:
            persist = es.enter_context(tc.tile_pool(name="persist", bufs=1))
            qT_t = persist.tile([128, NT], f32r, tag="qT")
            kT_t = persist.tile([128, NT], f32r, tag="kT")
            qcT_t = persist.tile([128, NT], f32r, tag="qcT")
            kcT_t = persist.tile([128, NTC], f32r, tag="kcT")
            vn_t = persist.tile([128, (NT // 128) * 130], f16, tag="vn")  # [v_h0|1|v_h1|1] per k-tile
            vcn_t = persist.tile([128, (NTC // 128) * 130], f16, tag="vcn")
            yT2_t = persist.tile([128, NT], f32r, tag="yT2")
            wp_t = persist.tile([128, C], f32r, tag="wp")
            bias_t = persist.tile([128, 6], f32, tag="bias")
            mask_t = persist.tile([128, 128], f16, tag="mask")

            nc.sync.dma_start(out=wp_t[:], in_=wp[:])
            nc.sync.dma_start(out=bias_t[:], in_=bias6[:])
            nc.sync.dma_start(out=mask_t[:], in_=maskd[:])

            ident_t = persist.tile([128, 128], f32r, tag="ident")
            nc.sync.dma_start(out=ident_t[:], in_=identd[:])
            vn_r = vn_t[:].rearrange("p (t c) -> p t c", c=130)
            nc.vector.memset(vn_r[:, :, 64:65], 1.0)
            nc.vector.memset(vn_r[:, :, 129:130], 1.0)
            vcn_r = vcn_t[:].rearrange("p (t c) -> p t c", c=130)
            nc.vector.memset(vcn_r[:, :, 64:65], 1.0)
            nc.vector.memset(vcn_r[:, :, 129:130], 1.0)
            zcolf = persist.tile([1, 128], f32, tag="zcolf")
            nc.vector.memset(zcolf[:], 0.0)
            zcol_t = persist.tile([1, 128], f32r, tag="zcol")
            nc.vector.tensor_copy(zcol_t[:], zcolf[:])
            zrowf = persist.tile([1, 512], f32, tag="zrowf")
            nc.vector.memset(zrowf[:], 0.0)
            zrow_t = persist.tile([1, 512], f32r, tag="zrow")
            nc.vector.tensor_copy(zrow_t[:], zrowf[:])

            # ---------------- Phase A: projections ----------------
            with ExitStack() as esa:
                wpool = esa.enter_context(tc.tile_pool(name="wpool", bufs=1))
                apool = esa.enter_context(tc.tile_pool(name="apool", bufs=2))
                vtpool = esa.enter_context(tc.tile_pool(name="vtpool", bufs=1))
                aps = esa.enter_context(tc.tile_pool(name="aps", bufs=3, space="PSUM"))
                apt = esa.enter_context(tc.tile_pool(name="apt", bufs=3, space="PSUM"))

                wq_t = wpool.tile([128, KT_X, CPC], f32r, tag="wq")
                wk_t = wpool.tile([128, KT_X, CPC], f32r, tag="wk")
                wv_t = wpool.tile([128, KT_X, CPC], f32r, tag="wv")
                wcq_t = wpool.tile([128, KT_X, CPC], f32r, tag="wcq")
                wck_t = wpool.tile([128, KT_C, CPC], f32r, tag="wck")
                wcv_t = wpool.tile([128, KT_C, CPC], f32r, tag="wcv")
                for wdram, wtile in ((wq, wq_t), (wk, wk_t), (wv, wv_t),
                                     (wcq, wcq_t), (wck, wck_t), (wcv, wcv_t)):
                    nc.sync.dma_start(
                        out=wtile[:],
                        in_=wdram.rearrange("(kt p) d -> p kt d", p=128))

                vT_t = vtpool.tile([128, NT], f32r, tag="vT")
                vcT_t = vtpool.tile([128, NTC], f32r, tag="vcT")

                def psum_evict(dst_slice, ps, bcol):
                    if zero_bias:
                        nc.vector.tensor_copy(dst_slice, ps[:])
                    else:
                        nc.vector.tensor_scalar_add(dst_slice, ps[:],
                                                    bias_t[:, bcol:bcol + 1])

                xT_r = xT.rearrange("(kt p) t -> p kt t", p=128)
                for ch in range(NCH):
                    xblk = apool.tile([128, KT_X, 512], f32r, tag="xblk")
                    nc.sync.dma_start(out=xblk[:], in_=xT_r[:, :, ch * 512:(ch + 1) * 512])
                    for wtile, dst, bcol in ((wq_t, qT_t, 0), (wk_t, kT_t, 1),
                                             (wv_t, vT_t, 2), (wcq_t, qcT_t, 3)):
                        ps = aps.tile([128, 512], f32, tag="aps")
                        for kt in range(KT_X):
                            nc.tensor.matmul(ps[:], wtile[:, kt, :], xblk[:, kt, :],
                                             start=(kt == 0), stop=(kt == KT_X - 1))
                        psum_evict(dst[:, ch * 512:(ch + 1) * 512], ps, bcol)

                cT_r = cT.rearrange("(kt p) t -> p kt t", p=128)
                for ch in range(NCHC):
                    cblk = apool.tile([128, KT_C, 512], f32r, tag="cblk")
                    nc.sync.dma_start(out=cblk[:], in_=cT_r[:, :, ch * 512:(ch + 1) * 512])
                    for wtile, dst, bcol in ((wck_t, kcT_t, 4), (wcv_t, vcT_t, 5)):
                        ps = aps.tile([128, 512], f32, tag="aps")
                        for kt in range(KT_C):
                            nc.tensor.matmul(ps[:], wtile[:, kt, :], cblk[:, kt, :],
                                             start=(kt == 0), stop=(kt == KT_C - 1))
                        psum_evict(dst[:, ch * 512:(ch + 1) * 512], ps, bcol)

                # Phase A2: vT/vcT -> natural token-major fp16 layout via PE transpose
                for tt in range(NT // 128):
                    pt = apt.tile([128, 128], f32r, tag="apt")
                    nc.tensor.transpose(pt[:], vT_t[:, tt * 128:(tt + 1) * 128], ident_t[:])
                    nc.vector.tensor_copy(vn_t[:, tt * 130:tt * 130 + 64], pt[:, 0:64])
                    nc.vector.tensor_copy(vn_t[:, tt * 130 + 65:tt * 130 + 129], pt[:, 64:128])
                for tt in range(NTC // 128):
                    pt = apt.tile([128, 128], f32r, tag="apt")
                    nc.tensor.transpose(pt[:], vcT_t[:, tt * 128:(tt + 1) * 128], ident_t[:])
                    nc.vector.tensor_copy(vcn_t[:, tt * 130:tt * 130 + 64], pt[:, 0:64])
                    nc.vector.tensor_copy(vcn_t[:, tt * 130 + 65:tt * 130 + 129], pt[:, 64:128])

            # ---------------- Phase B: attention ----------------
            with ExitStack() as esb:
                bpool = esb.enter_context(tc.tile_pool(name="bpool", bufs=2))
                rdpool = esb.enter_context(tc.tile_pool(name="rdpool", bufs=2, space="DRAM"))
                ypool = esb.enter_context(tc.tile_pool(name="ypool", bufs=3))
                expool = esb.enter_context(tc.tile_pool(name="expool", bufs=8))
                stps = esb.enter_context(tc.tile_pool(name="stps", bufs=2, space="PSUM"))
                yh0ps = esb.enter_context(tc.tile_pool(name="yh0ps", bufs=2, space="PSUM"))
                yh1ps = esb.enter_context(tc.tile_pool(name="yh1ps", bufs=2, space="PSUM"))

                def attn_part(b, qc, qlo, is_self):
                    """One softmax-attention accumulation (self or cross) for a
                    512-wide q chunk of batch b. Both heads' transposed score
                    tiles live in one [128,1024] two-bank PSUM tile so a single
                    exp serves both; V tiles carry a ones column so AV row 64
                    accumulates the softmax denominator. Returns per-head
                    normalized [64,512] tiles (ya, yb)."""
                    nkt = (4 * qc + 4) if is_self else KT_C
                    yh_0 = yh0ps.tile([65, 512], f32, tag="yh0")
                    yh_1 = yh1ps.tile([65, 512], f32, tag="yh1")
                    yh = (yh_0, yh_1)

                    pend = []
                    fidx = [0]

                    def flush_one():
                        ex, off, vsrc, vc0, vc1 = pend.pop(0)
                        first = fidx[0] == 0
                        last = fidx[0] == nkt - 1
                        fidx[0] += 1
                        nc.tensor.matmul(
                            yh[0][:, off:512],
                            vsrc[:, vc0:vc0 + 65],
                            ex[:, off:512],
                            start=first, stop=last)
                        nc.tensor.matmul(
                            yh[1][:, off:512],
                            vsrc[:, vc1:vc1 + 65],
                            ex[:, 512 + off:1024],
                            start=first, stop=last)

                    for kt in range(nkt):
                        if is_self:
                            crossing = kt >= 4 * qc
                            off = (kt - 4 * qc) * 128 if crossing else 0
                            klo = b * T + kt * 128
                            ksrc, qsrc, vsrc = kT_t, qT_t, vn_t
                            vbase = (b * KT_PER_B + kt) * 130
                        else:
                            crossing, off = False, 0
                            klo = b * TC + kt * 128
                            ksrc, qsrc, vsrc = kcT_t, qcT_t, vcn_t
                            vbase = (b * KT_C + kt) * 130
                        st = stps.tile([128, 1024], f32, tag="st")
                        nc.tensor.matmul(
                            st[:, off:512],
                            ksrc[0:64, klo:klo + 128],
                            qsrc[0:64, qlo + off:qlo + 512],
                            start=True, stop=True)
                        nc.tensor.matmul(
                            st[:, 512 + off:1024],
                            ksrc[64:128, klo:klo + 128],
                            qsrc[64:128, qlo + off:qlo + 512],
                            start=True, stop=True)
                        ex = expool.tile([128, 1024], f16, tag="ex")
                        nc.scalar.activation(ex[:, off:512], st[:, off:512],
                                             Exp, scale=SCALE)
                        nc.scalar.activation(ex[:, 512 + off:1024],
                                             st[:, 512 + off:1024],
                                             Exp, scale=SCALE)
                        if crossing:
                            nc.vector.tensor_mul(ex[:, off:off + 128],
                                                 ex[:, off:off + 128], mask_t[:])
                            nc.vector.tensor_mul(ex[:, 512 + off:512 + off + 128],
                                                 ex[:, 512 + off:512 + off + 128],
                                                 mask_t[:])
                        pend.append((ex, off, vsrc, vbase, vbase + 65))
                        if len(pend) > LOOKAHEAD:
                            flush_one()
                    while pend:
                        flush_one()

                    # denominators (row 64 of each head tile) -> DRAM -> [128,8]
                    # so the reciprocal uses all 128 DVE lanes -> broadcast back
                    dcp = bpool.tile([65, 1024], f32, tag="dcp")
                    nc.vector.tensor_copy(dcp[64:65, 0:512], yh[0][64:65, :])
                    nc.vector.tensor_copy(dcp[64:65, 512:1024], yh[1][64:65, :])
                    drd = rdpool.tile([1, 1024], f32, tag="drd")
                    nc.sync.dma_start(out=drd[:], in_=dcp[64:65, :])
                    dsb = bpool.tile([128, 8], f32, tag="dsb")
                    nc.sync.dma_start(
                        out=dsb[:], in_=drd[:].rearrange("a (p e) -> p (a e)", p=128))
                    rsb = bpool.tile([128, 8], f32, tag="rsb")
                    nc.vector.reciprocal(rsb[:], dsb[:])
                    rrd = rdpool.tile([1, 1024], f32, tag="rrd")
                    nc.sync.dma_start(
                        out=rrd[:].rearrange("a (p e) -> p (a e)", p=128), in_=rsb[:])
                    bca = bpool.tile([64, 512], f32, tag="bca")
                    nc.sync.dma_start(out=bca[:],
                                      in_=rrd[0:1, 0:512].to_broadcast((64, 512)))
                    bcb = bpool.tile([64, 512], f32, tag="bcb")
                    nc.sync.dma_start(out=bcb[:],
                                      in_=rrd[0:1, 512:1024].to_broadcast((64, 512)))
                    ya = ypool.tile([64, 512], f32, tag="ya")
                    nc.vector.tensor_mul(ya[:], yh[0][0:64, :], bca[:])
                    yb = ypool.tile([64, 512], f32, tag="yb")
                    nc.vector.tensor_mul(yb[:], yh[1][0:64, :], bcb[:])
                    return ya, yb

                for b in range(B):
                    for qc in range(QC_PER_B):
                        qlo = b * T + qc * 512
                        ya_c, yb_c = attn_part(b, qc, qlo, is_self=False)
                        ya_s, yb_s = attn_part(b, qc, qlo, is_self=True)
                        nc.vector.tensor_add(yT2_t[0:64, qlo:qlo + 512],
                                             ya_s[:], ya_c[:])
                        ybsum = ypool.tile([64, 512], f32r, tag="ybsum")
                        nc.vector.tensor_add(ybsum[:], yb_s[:], yb_c[:])
                        # partition shift rows 0-63 -> 64-127 via SBUF-SBUF DMA
                        nc.sync.dma_start(out=yT2_t[64:128, qlo:qlo + 512],
                                          in_=ybsum[:])

            # ---------------- Phase C: output projection ----------------
            with ExitStack() as esc:
                cpool = esc.enter_context(tc.tile_pool(name="cpool", bufs=4))
                cps = esc.enter_context(tc.tile_pool(name="cps", bufs=4, space="PSUM"))
                for tt in range(NT // 128):
                    for co in range(2):
                        po = cps.tile([128, 512], f32, tag="po")
                        nc.tensor.matmul(po[:],
                                         yT2_t[:, tt * 128:(tt + 1) * 128],
                                         wp_t[:, co * 512:(co + 1) * 512],
                                         start=True, stop=True)
                        so = cpool.tile([128, 512], f32, tag="so")
                        if co == 0:
                            nc.vector.tensor_copy(so[:], po[:])
                        else:
                            nc.scalar.copy(so[:], po[:])
                        nc.sync.dma_start(
                            out=out[tt * 128:(tt + 1) * 128, co * 512:(co + 1) * 512],
                            in_=so[:])

    nc.compile()
    return nc


_NC_CACHE = {}


def _get_nc(zero_bias=False):
    if zero_bias not in _NC_CACHE:
        _NC_CACHE[zero_bias] = _build(zero_bias)
    return _NC_CACHE[zero_bias]


def make_in_maps(x, cross_input, Wk, bk, Wq, bq, Wv, bv, Wck, bck, Wcq, bcq,
                 Wcv, bcv, Wp, bp):
    """Host-side shard + layout prep. Returns per-core input maps."""
    xT = _round_fp32r(np.asarray(x, np.float32).reshape(NT, C).T)
    cT = _round_fp32r(np.asarray(cross_input, np.float32).reshape(NTC, CC).T)
    mask = np.triu(np.ones((128, 128), np.float32)).astype(np.float16)  # 1 iff kk<=qq
    Wq, Wk, Wv = (np.asarray(w, np.float32) for w in (Wq, Wk, Wv))
    Wcq, Wck, Wcv = (np.asarray(w, np.float32) for w in (Wcq, Wck, Wcv))
    Wp = np.asarray(Wp, np.float32)
    in_maps = []
    for c in range(NCORES):
        sl = slice(c * CPC, (c + 1) * CPC)
        bias6 = np.stack([np.asarray(v, np.float32)[sl] for v in
                          (bq, bk, bv, bcq, bck, bcv)], axis=1)
        in_maps.append({
            "xT": xT, "cT": cT,
            "wq": _round_fp32r(Wq[:, sl]), "wk": _round_fp32r(Wk[:, sl]),
            "wv": _round_fp32r(Wv[:, sl]), "wcq": _round_fp32r(Wcq[:, sl]),
            "wck": _round_fp32r(Wck[:, sl]), "wcv": _round_fp32r(Wcv[:, sl]),
            "wp": _round_fp32r(Wp[sl, :]),
            "bias6": np.ascontiguousarray(bias6),
            "mask": mask,
            "ident": np.eye(128, dtype=np.float32),
        })
    return in_maps


def kernel(**inputs):
    in_maps = make_in_maps(**inputs)
    zb = all(not np.any(np.asarray(inputs[k])) for k in
             ("bq", "bk", "bv", "bcq", "bck", "bcv"))
    nc = _get_nc(zero_bias=zb)
    res = run_bass_kernel_spmd(nc, in_maps, list(range(NCORES)))
    acc = np.zeros((NT, C), np.float64)
    for c in range(NCORES):
        acc += res.results[c]["out"]
    acc += np.asarray(inputs["bp"], np.float32)
    return acc.reshape(B, T, C).astype(np.float32)


if __name__ == "__main__":
    nc = _get_nc()
    print("build + compile OK")
